# revision 19
# baseline (speedup 1.0000x reference)
"""Trainium2 Bass kernel for nn_MiMoV2FlashBlock (GQA attention block with
partial RoPE and attention-sink softmax), sharded across 8 NeuronCores.

Sharding: tensor-parallel over heads. Core i computes q-heads 4i..4i+3 and
kv-head i, plus the matching input-dim shard of the o-projection. Each core
emits a partial [S, HID] output (bf16); the host sums the 8 partials in fp64.

Precision strategy (strip-hybrid): query rows < 512 ("strip 0") take an
exact path (fp32 projections, bf16 q/k, fp32r attention); rows >= 512 run on
the fp8e4m3 DoubleRow path (4x PE throughput) where softmax averaging over a
large attention support washes out quantization noise:
  - projections for positions >= 512: fp8 hs x fp8 W, DoubleRow pairs.
  - scores: fp8 q/k split into two 64-partition d-planes, DoubleRow.
  - probs: exp -> fp8 directly; denominator via all-ones DoubleRow matmul of
    the SAME fp8 probs (quantization cancels in the ratio).
  - PV: DoubleRow with v in two fp8 terms (hi + residual) for accuracy.
  - causal masking for fp8 blocks: identity-DoubleRow accumulate of the mask
    (in raw-score units, clamped to +-240 = e4m3 max finite) into the scores
    PSUM before exp; exp then flushes masked entries to 0.
o-projection runs in bf16. RoPE's rotate-half uses an SBUF->SBUF DMA
partition swap (sign folded into the sin table) instead of PE matmuls.
Validated end-to-end against the fp32 reference: rel err ~8e-3 (gate 2e-2).
"""

import sys

for _p in ("/opt/trn_rl_repo",):
    if _p not in sys.path:
        sys.path.insert(0, _p)

import numpy as np
import ml_dtypes

import concourse.bass as bass  # noqa: E402
import concourse.mybir as mybir  # noqa: E402
import concourse.tile as tile  # noqa: E402
from concourse import bacc  # noqa: E402
from concourse import hw_specs as _hw_specs  # noqa: E402
from concourse.bass_utils import run_bass_kernel_spmd  # noqa: E402

# Pin activation-table selection to the set containing Exp, Ln and Copy so the
# scheduler never pays a mid-kernel ACT_TABLE_LOAD.
_orig_gat = _hw_specs.get_activation_tables


def _pinned_act_tables(arch):
    full = _orig_gat(arch)
    return {
        k: (v if k == "natural_log_exp_and_others" else set())
        for k, v in full.items()
    }


bacc.get_activation_tables = _pinned_act_tables

F32 = mybir.dt.float32
F32R = mybir.dt.float32r
BF16 = mybir.dt.bfloat16
F8 = mybir.dt.float8e4
AF = mybir.ActivationFunctionType
ALU = mybir.AluOpType
DR = mybir.MatmulPerfMode.DoubleRow
E4 = ml_dtypes.float8_e4m3
BF = ml_dtypes.bfloat16

B, S, HID = 1, 2048, 2048
H, KVH, D, VD = 32, 8, 128, 128
R = 64
N_CORES = 8
QH_L = H // N_CORES          # 4 local q heads per core
SCALE = float(D) ** -0.5

NT = S // 128                # 16 k tiles
NH = HID // 128              # 16 hidden tiles
NJ = S // 512                # 4 q strips
DT = (QH_L * VD) // 128      # 4 o-proj contraction tiles

SKIP, PLAIN = -1, -2
_DBG = False

_cache: dict = {}


def _build(sched):
    """sched[j][kt] in {SKIP, PLAIN, idx>=0}. Strip 0 mask tiles are fp32
    (idx into msk0); strips >=1 are fp8 plane tiles (idx into msk8)."""
    n0 = max(1 + max((sched[0][kt] for kt in range(NT)), default=-1), 1)
    n8 = max(max((sched[j][kt] for kt in range(NT)), default=-1)
             for j in range(1, NJ)) + 1
    n8 = max(n8, 1)

    nc = bacc.Bacc(None, target_bir_lowering=False)

    hsx0_h = nc.dram_tensor("hsx0", [128, NH, 512], BF16, kind="ExternalInput")
    hs8_h = nc.dram_tensor("hs8", [128, NH // 2, 2, S - 512], F8,
                           kind="ExternalInput")
    wqf_h = nc.dram_tensor("wqf", [128, NH, QH_L * 128], BF16,
                           kind="ExternalInput")
    wkf_h = nc.dram_tensor("wkf", [128, NH, 128], BF16, kind="ExternalInput")
    wvf_h = nc.dram_tensor("wvf", [128, NH, 128], BF16, kind="ExternalInput")
    wq8_h = nc.dram_tensor("wq8", [128, NH // 2, 2, QH_L * 128], F8,
                           kind="ExternalInput")
    wk8_h = nc.dram_tensor("wk8", [128, NH // 2, 2, 128], F8,
                           kind="ExternalInput")
    wv8_h = nc.dram_tensor("wv8", [128, NH // 2, 2, 128], F8,
                           kind="ExternalInput")
    wob_h = nc.dram_tensor("wob", [128, DT, HID], BF16, kind="ExternalInput")
    csb_h = nc.dram_tensor("csb", [64, S], BF16, kind="ExternalInput")
    snb_h = nc.dram_tensor("snb", [64, S], BF16, kind="ExternalInput")
    msk0_h = nc.dram_tensor("msk0", [n0, 128, 512], BF16, kind="ExternalInput")
    msk8_h = nc.dram_tensor("msk8", [n8, 64, 2, 512], F8, kind="ExternalInput")
    id8_h = nc.dram_tensor("id8", [64, 2, 128], F8, kind="ExternalInput")
    ones8_h = nc.dram_tensor("ones8", [128, 2, 32], F8, kind="ExternalInput")
    identr_h = nc.dram_tensor("identr", [128, 128], F32R, kind="ExternalInput")
    identb_h = nc.dram_tensor("identb", [128, 128], BF16, kind="ExternalInput")
    onesf_h = nc.dram_tensor("onesf", [128, 1], F32R, kind="ExternalInput")
    sink_h = nc.dram_tensor("sinkexp", [1, QH_L], F32, kind="ExternalInput")
    out_h = nc.dram_tensor("out", [S, HID], BF16, kind="ExternalOutput")
    if _DBG:
        dbg_qk8_h = nc.dram_tensor("dbg_qk8", [64, 2, S], F8,
                                   kind="ExternalOutput")
        dbg_kk8_h = nc.dram_tensor("dbg_kk8", [64, 2, S], F8,
                                   kind="ExternalOutput")
        dbg_sc_h = nc.dram_tensor("dbg_sc", [128, 2, 512], F32,
                                  kind="ExternalOutput")
        dbg_pr_h = nc.dram_tensor("dbg_pr", [128, 2, 512], F8,
                                  kind="ExternalOutput")
        dbg_dn_h = nc.dram_tensor("dbg_dn", [32, 512], F32,
                                  kind="ExternalOutput")
        dbg_v_h = nc.dram_tensor("dbg_v", [128, NT, VD], F8,
                                 kind="ExternalOutput")

    lp = nc.allow_low_precision(reason="hybrid fp8/bf16 kernel, validated")
    lp.__enter__()

    with tile.TileContext(nc) as tc:
        with (
            tc.tile_pool(name="consts", bufs=1) as cpool,
            tc.tile_pool(name="big", bufs=1) as bigpool,
            tc.tile_pool(name="small", bufs=3) as spool,
        ):
            # ---- constants (gpsimd SWDGE queue; tiny) ----
            identr = cpool.tile([128, 128], F32R)
            nc.gpsimd.dma_start(identr[:], identr_h[:])
            identb = cpool.tile([128, 128], BF16)
            nc.gpsimd.dma_start(identb[:], identb_h[:])
            onesf = cpool.tile([128, 1], F32R)
            nc.gpsimd.dma_start(onesf[:], onesf_h[:])
            ones8 = cpool.tile([128, 2, 32], F8)
            nc.gpsimd.dma_start(ones8[:], ones8_h[:])
            id8 = cpool.tile([64, 2, 128], F8)
            nc.gpsimd.dma_start(id8[:], id8_h[:])
            sinkexp = cpool.tile([1, QH_L], F32)
            nc.gpsimd.dma_start(sinkexp[:], sink_h[:])
            csb = cpool.tile([64, S], BF16)
            nc.gpsimd.dma_start(csb[:], csb_h[:])
            snb = cpool.tile([64, S], BF16)
            nc.gpsimd.dma_start(snb[:], snb_h[:])

            # ---- resident activations ----
            qTb = [bigpool.tile([128, 512], BF16, tag=f"qTb{h}",
                                name=f"qTb{h}") for h in range(QH_L)]
            kTb = bigpool.tile([128, 512], BF16, tag="kTb")
            qk8 = [bigpool.tile([64, 2, S], F8, tag=f"qk8_{h}",
                                name=f"qk8_{h}") for h in range(QH_L)]
            kk8 = bigpool.tile([64, 2, S], F8, tag="kk8")
            vsbf = bigpool.tile([128, 4, VD], F32R, tag="vsbf")
            vsb8h = bigpool.tile([128, NT, VD], F8, tag="vsb8h")
            vsb8l = bigpool.tile([128, NT, VD], F8, tag="vsb8l")
            oTb = [bigpool.tile([128, S], BF16, tag=f"oTb{h}",
                                name=f"oTb{h}") for h in range(QH_L)]
            wob = bigpool.tile([128, DT, HID], BF16, tag="wob")

            # ================= phase A =================
            # resident inputs; fine chunks early so compute starts ASAP
            hsx0b = bigpool.tile([128, NH, 512], BF16, tag="hsx0b")
            for ch in range(4):
                nc.sync.dma_start(hsx0b[:, 4 * ch:4 * (ch + 1), :],
                                  hsx0_h[:, 4 * ch:4 * (ch + 1), :])
            hs8sb = bigpool.tile([128, NH // 2, 2, S - 512], F8, tag="hs8sb")
            for jc in range(2):
                nc.sync.dma_start(hs8sb[:, :, :, 512 * jc:512 * (jc + 1)],
                                  hs8_h[:, :, :, 512 * jc:512 * (jc + 1)])
            wqfb = bigpool.tile([128, NH, QH_L * 128], BF16, tag="wqfb")
            for t in range(4):
                nc.scalar.dma_start(wqfb[:, t, :], wqf_h[:, t, :])
            for ch in range(1, 4):
                nc.scalar.dma_start(wqfb[:, 4 * ch:4 * (ch + 1), :],
                                    wqf_h[:, 4 * ch:4 * (ch + 1), :])
            nc.scalar.dma_start(hs8sb[:, :, :, 1024:1536],
                                hs8_h[:, :, :, 1024:1536])
            wkfb = bigpool.tile([128, NH, 128], BF16, tag="wkfb")
            nc.gpsimd.dma_start(wkfb[:], wkf_h[:])
            wvfb = bigpool.tile([128, NH, 128], BF16, tag="wvfb")
            nc.gpsimd.dma_start(wvfb[:], wvf_h[:])
            msk0 = []
            for m in range(n0):
                t = cpool.tile([128, 512], BF16, tag=f"m0_{m}", name=f"m0_{m}")
                nc.gpsimd.dma_start(t[:], msk0_h[m])
                msk0.append(t)
            msk8 = []
            for m in range(n8):
                t = cpool.tile([64, 2, 512], F8, tag=f"m8_{m}", name=f"m8_{m}")
                nc.gpsimd.dma_start(t[:], msk8_h[m])
                msk8.append(t)
            with (
                tc.tile_pool(name="dstscr", bufs=3) as dpool,
                tc.tile_pool(name="psA", bufs=7, space="PSUM") as psA,
                tc.tile_pool(name="psX", bufs=1, space="PSUM") as psX,
            ):
                def rope_and_store(pp, hh, j, jsl):
                    """pp: psum [128,512] fp32 of q-head hh (or k if hh==4).
                    Writes bf16 (strip0) / fp8 planes (j>=1)."""
                    if j == 0:
                        dst = qTb[hh] if hh < QH_L else kTb
                        dsl = slice(0, 512)
                    else:
                        dst = dpool.tile([128, 512], BF16, tag="dst",
                                         name=f"dst{j}_{hh}")
                        dsl = slice(0, 512)
                    nc.scalar.copy(dst[:, dsl], pp[:])
                    swp = spool.tile([64, 512], BF16, tag="swp")
                    nc.sync.dma_start(swp[0:32, :], dst[32:64, dsl])
                    nc.sync.dma_start(swp[32:64, :], dst[0:32, dsl])
                    m1 = spool.tile([64, 512], BF16, tag="m1")
                    nc.vector.tensor_tensor(m1[:], dst[0:64, dsl],
                                            csb[:, jsl], ALU.mult)
                    m2 = spool.tile([64, 512], BF16, tag="m2")
                    nc.vector.tensor_tensor(m2[:], swp[:], snb[:, jsl],
                                            ALU.mult)
                    if j == 0:
                        nc.vector.tensor_tensor(dst[0:64, dsl], m1[:], m2[:],
                                                ALU.add)
                        if hh == QH_L:   # k head: also fp8 planes for keys<512
                            nc.vector.tensor_copy(kk8[:, 0, jsl],
                                                  dst[0:64, dsl])
                            scr = spool.tile([128, 512], F8, tag="scr")
                            nc.vector.tensor_copy(scr[64:128, :],
                                                  dst[64:128, dsl])
                            nc.sync.dma_start(kk8[:, 1, jsl], scr[64:128, :])
                    else:
                        t8 = qk8[hh] if hh < QH_L else kk8
                        nc.vector.tensor_tensor(t8[:, 0, jsl], m1[:], m2[:],
                                                ALU.add)
                        scr = spool.tile([128, 512], F8, tag="scr")
                        nc.vector.tensor_copy(scr[64:128, :], dst[64:128, dsl])
                        nc.sync.dma_start(t8[:, 1, jsl], scr[64:128, :])

                def v_store(pp, j):
                    vt = dpool.tile([128, 512], F32R, tag="vt",
                                    name=f"vt{j}")
                    nc.scalar.copy(vt[:], pp[:])
                    for st in range(4):
                        kt = 4 * j + st
                        tr = psX.tile([128, 128], F32R, tag="tr",
                                      name=f"tr{j}_{st}")
                        nc.tensor.transpose(
                            tr[:], vt[:, st * 128:(st + 1) * 128], identr[:]
                        )
                        if j == 0:
                            nc.vector.tensor_copy(vsbf[:, kt, :], tr[:])
                        nc.vector.tensor_copy(vsb8h[:, kt, :], tr[:])
                        tm = spool.tile([128, 128], F32, tag="vtm")
                        nc.vector.tensor_tensor(tm[:], tr[:],
                                                vsb8h[:, kt, :], ALU.subtract)
                        nc.vector.tensor_copy(vsb8l[:, kt, :], tm[:])

                # ---- strip 0: fp32r ----
                jsl0 = slice(0, 512)
                # pass 1: q heads (needs only wqfb chunks + hsx0b)
                pp0 = [psA.tile([128, 512], F32, tag="pp", name=f"pp0_{g}")
                       for g in range(QH_L)]
                for t in range(NH):
                    for g in range(QH_L):
                        nc.tensor.matmul(
                            pp0[g][:], wqfb[:, t, g * 128:(g + 1) * 128],
                            hsx0b[:, t, :],
                            start=(t == 0), stop=(t == NH - 1))
                for hh in range(QH_L):
                    rope_and_store(pp0[hh], hh, 0, jsl0)
                # pass 2: k and v
                ppk = psA.tile([128, 512], F32, tag="pp", name="pp0_k")
                ppv = psA.tile([128, 512], F32, tag="pp", name="pp0_v")
                for t in range(NH):
                    nc.tensor.matmul(ppk[:], wkfb[:, t, :], hsx0b[:, t, :],
                                     start=(t == 0), stop=(t == NH - 1))
                    nc.tensor.matmul(ppv[:], wvfb[:, t, :], hsx0b[:, t, :],
                                     start=(t == 0), stop=(t == NH - 1))
                rope_and_store(ppk, QH_L, 0, jsl0)
                v_store(ppv, 0)

                # ---- strips 1..3: fp8 DoubleRow ----
                wq8 = bigpool.tile([128, NH // 2, 2, QH_L * 128], F8,
                                   tag="wq8")
                nc.gpsimd.dma_start(wq8[:], wq8_h[:])
                wk8 = bigpool.tile([128, NH // 2, 2, 128], F8, tag="wk8")
                nc.gpsimd.dma_start(wk8[:], wk8_h[:])
                wv8 = bigpool.tile([128, NH // 2, 2, 128], F8, tag="wv8")
                nc.gpsimd.dma_start(wv8[:], wv8_h[:])
                # o-proj weights: needed only in phase C; load in background
                nc.gpsimd.dma_start(wob[:], wob_h[:])
                for j in range(1, NJ):
                    jsl = slice(512 * j, 512 * (j + 1))
                    rsl = slice(512 * (j - 1), 512 * j)  # hs8 is offset by 512
                    pp = [psA.tile([128, 512], F32, tag="pp",
                                   name=f"pp{j}_{g}")
                          for g in range(QH_L + 2)]
                    for t in range(NH // 2):
                        for g in range(QH_L + 2):
                            lhsT = (wq8[:, t, :, g * 128:(g + 1) * 128]
                                    if g < QH_L else
                                    (wk8[:, t, :, :] if g == QH_L
                                     else wv8[:, t, :, :]))
                            nc.tensor.matmul(pp[g][:], lhsT,
                                             hs8sb[:, t, :, rsl],
                                             start=(t == 0),
                                             stop=(t == NH // 2 - 1),
                                             perf_mode=DR)
                    for hh in range(QH_L + 1):
                        rope_and_store(pp[hh], hh, j, jsl)
                    v_store(pp[QH_L + 1], j)

            if _DBG:
                nc.sync.dma_start(dbg_qk8_h[:], qk8[0][:])
                nc.sync.dma_start(dbg_kk8_h[:], kk8[:])
                nc.sync.dma_start(dbg_v_h[:], vsb8h[:])

            # ================= phases B + C =================
            with (
                tc.tile_pool(name="pr8p", bufs=4) as pr8pool,
                tc.tile_pool(name="prfp", bufs=3) as prfpool,
                tc.tile_pool(name="osbp", bufs=2) as osbpool,
                tc.tile_pool(name="psSC", bufs=2, space="PSUM") as psSC,
                tc.tile_pool(name="psO", bufs=2, space="PSUM") as psO,
                tc.tile_pool(name="psDN", bufs=2, space="PSUM") as psDN,
            ):
                def emit_tail(tail):
                    dnrow, oacc, h, jsl_ = tail
                    dns = spool.tile([1, 512], F32, tag="dns")
                    nc.vector.tensor_scalar_add(dns[:], dnrow,
                                                sinkexp[0:1, h:h + 1])
                    recip = spool.tile([1, 512], F32, tag="recip")
                    nc.vector.reciprocal_approx_fast(recip[:], dns[:])
                    bc = spool.tile([128, 512], F32, tag="bc", bufs=2)
                    nc.gpsimd.partition_broadcast(bc[:], recip[:])
                    nc.vector.tensor_tensor(oTb[h][:, jsl_], oacc[:], bc[:],
                                            ALU.mult)

                def emit_b_strip(j):
                    jsl = slice(512 * j, 512 * (j + 1))
                    ktmax = 4 if j == 0 else NT
                    kts = [kt for kt in range(ktmax) if sched[j][kt] != SKIP]
                    prs = [kts[i:i + 2] for i in range(0, len(kts), 2)]
                    for pair_i, (ha, hb) in enumerate(((0, 1), (2, 3))):
                        st = {}
                        for h in (ha, hb):
                            st[h] = {
                                "oacc": psO.tile([128, 512], F32, tag="oacc",
                                                 name=f"oacc{j}_{h}"),
                                "dn": psDN.tile([32, 512], F32, tag="dn",
                                                name=f"dn{j}_{pair_i}_{h}"),
                                "first": True,
                            }

                        def emit_pv(pend, last):
                            grp, prt = pend
                            for h in (ha, hb):
                                pr = prt[h]
                                if j == 0:
                                    for z, kt in enumerate(grp):
                                        lst = last and z == len(grp) - 1
                                        nc.tensor.matmul(
                                            st[h]["oacc"][:],
                                            vsbf[:, kt, :], pr[:, z, :],
                                            start=st[h]["first"], stop=lst)
                                        nc.tensor.matmul(
                                            st[h]["dn"][0:1, :],
                                            onesf[:], pr[:, z, :],
                                            start=st[h]["first"], stop=lst,
                                            skip_group_check=True)
                                        st[h]["first"] = False
                                elif len(grp) == 2 and grp[1] == grp[0] + 1:
                                    k0 = grp[0]
                                    nc.tensor.matmul(
                                        st[h]["oacc"][:],
                                        vsb8h[:, k0:k0 + 2, :], pr[:],
                                        start=st[h]["first"], stop=False,
                                        perf_mode=DR)
                                    nc.tensor.matmul(
                                        st[h]["oacc"][:],
                                        vsb8l[:, k0:k0 + 2, :], pr[:],
                                        start=False, stop=last,
                                        perf_mode=DR)
                                    nc.tensor.matmul(
                                        st[h]["dn"][0:32, :],
                                        ones8[:], pr[:],
                                        start=st[h]["first"], stop=last,
                                        perf_mode=DR, skip_group_check=True)
                                    st[h]["first"] = False
                                else:
                                    for z, kt in enumerate(grp):
                                        lst = last and z == len(grp) - 1
                                        nc.tensor.matmul(
                                            st[h]["oacc"][:],
                                            vsb8h[:, kt, :], pr[:, z, :],
                                            start=st[h]["first"], stop=False)
                                        nc.tensor.matmul(
                                            st[h]["oacc"][:],
                                            vsb8l[:, kt, :], pr[:, z, :],
                                            start=False, stop=lst)
                                        nc.tensor.matmul(
                                            st[h]["dn"][0:32, :],
                                            ones8[:, 0, :], pr[:, z, :],
                                            start=st[h]["first"], stop=lst,
                                            skip_group_check=True)
                                        st[h]["first"] = False

                        pend = None
                        for pi, grp in enumerate(prs):
                            prt = {}
                            for h in (ha, hb):
                                sc2 = psSC.tile([128, 2, 512], F32,
                                                tag="sc2",
                                                name=f"sc{j}_{pair_i}_{h}_{pi}")
                                for z, kt in enumerate(grp):
                                    ksl = slice(kt * 128, (kt + 1) * 128)
                                    code = sched[j][kt]
                                    if j == 0:
                                        nc.tensor.matmul(
                                            sc2[:, z, :], kTb[:, ksl],
                                            qTb[h][:], start=True,
                                            stop=(code < 0))
                                        if code >= 0:
                                            nc.tensor.matmul(
                                                sc2[:, z, :], identb[:],
                                                msk0[code][:],
                                                start=False, stop=True)
                                    else:
                                        nc.tensor.matmul(
                                            sc2[:, z, :], kk8[:, :, ksl],
                                            qk8[h][:, :, jsl],
                                            start=True, stop=(code < 0),
                                            perf_mode=DR)
                                        if code >= 0:
                                            nc.tensor.matmul(
                                                sc2[:, z, :], id8[:],
                                                msk8[code][:],
                                                start=False, stop=True,
                                                perf_mode=DR)
                                if j == 0:
                                    pr = prfpool.tile([128, 2, 512], F32R,
                                                      tag="prf")
                                else:
                                    pr = pr8pool.tile([128, 2, 512], F8,
                                                      tag="pr8")
                                nc.scalar.activation(
                                    pr[:, 0:len(grp), :],
                                    sc2[:, 0:len(grp), :],
                                    AF.Exp, scale=SCALE)
                                prt[h] = pr
                            if pend is not None:
                                emit_pv(pend, last=False)
                            pend = (grp, prt)
                        if pend is not None:
                            emit_pv(pend, last=True)
                        for h in (ha, hb):
                            if st[h]["first"]:   # no valid kt at all
                                nc.vector.memset(oTb[h][:, jsl], 0.0)
                                continue
                            emit_tail((st[h]["dn"][0:1, :],
                                       st[h]["oacc"], h, jsl))

                emit_b_strip(0)
                emit_b_strip(1)
                emit_b_strip(2)
                emit_b_strip(3)

            # phase C after B pools close: use a wide PSUM pool
            with (
                tc.tile_pool(name="osb2", bufs=3) as osbpool,
                tc.tile_pool(name="psC2", bufs=6, space="PSUM") as psC2,
            ):
                for qt in range(0, 16):
                    qsl = slice(qt * 128, (qt + 1) * 128)
                    osb = osbpool.tile([128, HID], BF16, tag="osb",
                                       name=f"osbf{qt}")
                    for hc in range(HID // 512):
                        hsl = slice(hc * 512, (hc + 1) * 512)
                        oc = psC2.tile([128, 512], F32, tag="oc")
                        for t in range(DT):
                            nc.tensor.matmul(
                                oc[:], oTb[t][:, qsl], wob[:, t, hsl],
                                start=(t == 0), stop=(t == DT - 1))
                        if hc % 2 == 0:
                            nc.scalar.copy(osb[:, hsl], oc[:])
                        else:
                            nc.vector.tensor_copy(osb[:, hsl], oc[:])
                    if qt % 2 == 0:
                        nc.sync.dma_start(out_h[qsl, :], osb[:])
                    else:
                        nc.gpsimd.dma_start(out_h[qsl, :], osb[:])

    lp.__exit__(None, None, None)
    nc.compile()
    return nc


def _classify_mask(mask):
    """Classify 512x128 blocks (strip j, k tile kt). Strip 0 gets fp32 mask
    tiles in raw-score units (mask/SCALE); strips >=1 get fp8 plane tiles."""
    sched = [[PLAIN] * NT for _ in range(NJ)]
    tiles0, seen0 = [], {}
    tiles8, seen8 = [], {}
    for j in range(NJ):
        for kt in range(NT):
            blk = mask[512 * j:512 * (j + 1), 128 * kt:128 * (kt + 1)]
            if np.all(blk <= -1e8):
                sched[j][kt] = SKIP
            elif not blk.any():
                sched[j][kt] = PLAIN
            else:
                key = blk.tobytes()
                if j == 0:
                    idx = seen0.get(key)
                    if idx is None:
                        idx = len(tiles0)
                        seen0[key] = idx
                        tiles0.append(
                            np.ascontiguousarray(blk.T / SCALE,
                                                 dtype=np.float32))
                    sched[j][kt] = idx
                else:
                    idx = seen8.get(key)
                    if idx is None:
                        idx = len(tiles8)
                        seen8[key] = idx
                        t = np.clip(blk.T / SCALE, -240.0, 240.0)
                        tiles8.append(
                            np.ascontiguousarray(
                                t.reshape(2, 64, 512).transpose(1, 0, 2)
                            ).astype(E4))
                    sched[j][kt] = idx
    m0 = (np.stack(tiles0) if tiles0
          else np.zeros((1, 128, 512), np.float32))
    m8 = (np.stack(tiles8) if tiles8
          else np.zeros((1, 64, 2, 512), E4))
    return sched, m0, m8


def _pt_layout(a, p=128):
    """[T*p, M] -> [p, T, M] partition-major tiling along the first axis."""
    t = a.shape[0] // p
    return np.ascontiguousarray(
        a.reshape(t, p, a.shape[1]).transpose(1, 0, 2), dtype=np.float32
    )


def kernel(**inputs):
    hs = np.asarray(inputs["hidden_states"], dtype=np.float32)[0]
    cos = np.asarray(inputs["cos"], dtype=np.float32)[0]
    sin = np.asarray(inputs["sin"], dtype=np.float32)[0]
    mask = np.asarray(inputs["attention_mask"], dtype=np.float32)[0, 0]
    Wq = np.asarray(inputs["Wq"], dtype=np.float32)
    Wk = np.asarray(inputs["Wk"], dtype=np.float32)
    Wv = np.asarray(inputs["Wv"], dtype=np.float32)
    Wo = np.asarray(inputs["Wo"], dtype=np.float32)
    sink = np.asarray(inputs["sink_bias"], dtype=np.float32)

    sched, m0, m8 = _classify_mask(mask)
    key = tuple(tuple(r) for r in sched)
    if key not in _cache:
        _cache[key] = _build(sched)
    nc = _cache[key]

    hsx = _pt_layout(hs.T)                           # [128, NH, S] fp32
    hsx0 = np.ascontiguousarray(hsx[:, :, 0:512]).astype(BF)
    hs8 = np.ascontiguousarray(
        hsx[:, :, 512:].reshape(128, NH // 2, 2, S - 512).astype(E4))

    csT = np.ascontiguousarray(cos.T)                # [64, S]
    snT = np.ascontiguousarray(sin.T).copy()
    snT[0:32] = -snT[0:32]                           # fold rotate-half sign
    csb = csT.astype(BF)
    snb = snT.astype(BF)

    id8 = np.zeros((64, 2, 128), E4)
    for i in range(64):
        id8[i, 0, i] = 1.0
        id8[i, 1, 64 + i] = 1.0
    ones8 = np.ones((128, 2, 32), E4)
    identr = np.eye(128, dtype=np.float32)
    onesf = np.ones((128, 1), np.float32)

    common = {
        "hsx0": hsx0, "hs8": hs8, "csb": csb, "snb": snb,
        "msk0": m0.astype(BF), "msk8": m8, "id8": id8, "ones8": ones8,
        "identr": identr, "identb": identr.astype(BF), "onesf": onesf,
        "wob": None,
    }

    in_maps = []
    for i in range(N_CORES):
        wqf = _pt_layout(np.ascontiguousarray(Wq[i * 512:(i + 1) * 512].T))
        wkf = _pt_layout(np.ascontiguousarray(Wk[i * 128:(i + 1) * 128].T))
        wvf = _pt_layout(np.ascontiguousarray(Wv[i * 128:(i + 1) * 128].T))
        wof = _pt_layout(np.ascontiguousarray(Wo[:, i * 512:(i + 1) * 512].T))
        se = np.exp(sink[i * QH_L:(i + 1) * QH_L]).reshape(1, QH_L)
        m = dict(common)
        m["wqf"] = wqf.astype(BF)
        m["wkf"] = wkf.astype(BF)
        m["wvf"] = wvf.astype(BF)
        m["wq8"] = np.ascontiguousarray(
            wqf.reshape(128, NH // 2, 2, 512)).astype(E4)
        m["wk8"] = np.ascontiguousarray(
            wkf.reshape(128, NH // 2, 2, 128)).astype(E4)
        m["wv8"] = np.ascontiguousarray(
            wvf.reshape(128, NH // 2, 2, 128)).astype(E4)
        m["wob"] = wof.astype(BF)
        m["sinkexp"] = np.ascontiguousarray(se, dtype=np.float32)
        in_maps.append(m)

    global _last
    _last = (nc, in_maps)
    res = run_bass_kernel_spmd(nc, in_maps, list(range(N_CORES)))
    out = np.zeros((S, HID), np.float64)
    for i in range(N_CORES):
        out += res.results[i]["out"].astype(np.float64)
    out = out.astype(np.float32).reshape(B, S, HID)
    if not np.isfinite(out).all():
        raise FloatingPointError(
            "kernel produced non-finite values; inputs outside the "
            "validated regime for the no-max-pass softmax"
        )
    return out


# revision 20
# speedup vs baseline: 1.0087x; 1.0087x over previous
"""Trainium2 Bass kernel for nn_MiMoV2FlashBlock (GQA attention block with
partial RoPE and attention-sink softmax), sharded across 8 NeuronCores.

Sharding: tensor-parallel over heads. Core i computes q-heads 4i..4i+3 and
kv-head i, plus the matching input-dim shard of the o-projection. Each core
emits a partial [S, HID] output (bf16); the host sums the 8 partials in fp64.

Precision strategy (strip-hybrid): query rows < 512 ("strip 0") take an
exact path (fp32 projections, bf16 q/k, fp32r attention); rows >= 512 run on
the fp8e4m3 DoubleRow path (4x PE throughput) where softmax averaging over a
large attention support washes out quantization noise:
  - projections for positions >= 512: fp8 hs x fp8 W, DoubleRow pairs.
  - scores: fp8 q/k split into two 64-partition d-planes, DoubleRow.
  - probs: exp -> fp8 directly; denominator via all-ones DoubleRow matmul of
    the SAME fp8 probs (quantization cancels in the ratio).
  - PV: DoubleRow with v in two fp8 terms (hi + residual) for accuracy.
  - causal masking for fp8 blocks: identity-DoubleRow accumulate of the mask
    (in raw-score units, clamped to +-240 = e4m3 max finite) into the scores
    PSUM before exp; exp then flushes masked entries to 0.
o-projection runs in bf16. RoPE's rotate-half uses an SBUF->SBUF DMA
partition swap (sign folded into the sin table) instead of PE matmuls.
Validated end-to-end against the fp32 reference: rel err ~8e-3 (gate 2e-2).
"""

import sys

for _p in ("/opt/trn_rl_repo",):
    if _p not in sys.path:
        sys.path.insert(0, _p)

import numpy as np
import ml_dtypes

import concourse.bass as bass  # noqa: E402
import concourse.mybir as mybir  # noqa: E402
import concourse.tile as tile  # noqa: E402
from concourse import bacc  # noqa: E402
from concourse import hw_specs as _hw_specs  # noqa: E402
from concourse.bass_utils import run_bass_kernel_spmd  # noqa: E402

# Pin activation-table selection to the set containing Exp, Ln and Copy so the
# scheduler never pays a mid-kernel ACT_TABLE_LOAD.
_orig_gat = _hw_specs.get_activation_tables


def _pinned_act_tables(arch):
    full = _orig_gat(arch)
    return {
        k: (v if k == "natural_log_exp_and_others" else set())
        for k, v in full.items()
    }


bacc.get_activation_tables = _pinned_act_tables

F32 = mybir.dt.float32
F32R = mybir.dt.float32r
BF16 = mybir.dt.bfloat16
F8 = mybir.dt.float8e4
AF = mybir.ActivationFunctionType
ALU = mybir.AluOpType
DR = mybir.MatmulPerfMode.DoubleRow
E4 = ml_dtypes.float8_e4m3
BF = ml_dtypes.bfloat16

B, S, HID = 1, 2048, 2048
H, KVH, D, VD = 32, 8, 128, 128
R = 64
N_CORES = 8
QH_L = H // N_CORES          # 4 local q heads per core
SCALE = float(D) ** -0.5

NT = S // 128                # 16 k tiles
NH = HID // 128              # 16 hidden tiles
NJ = S // 512                # 4 q strips
DT = (QH_L * VD) // 128      # 4 o-proj contraction tiles

SKIP, PLAIN = -1, -2
_DBG = False

_cache: dict = {}


def _build(sched):
    """sched[j][kt] in {SKIP, PLAIN, idx>=0}. Strip 0 mask tiles are fp32
    (idx into msk0); strips >=1 are fp8 plane tiles (idx into msk8)."""
    n0 = max(1 + max((sched[0][kt] for kt in range(NT)), default=-1), 1)
    n8 = max(max((sched[j][kt] for kt in range(NT)), default=-1)
             for j in range(1, NJ)) + 1
    n8 = max(n8, 1)

    nc = bacc.Bacc(None, target_bir_lowering=False)

    hsx0_h = nc.dram_tensor("hsx0", [128, NH, 512], BF16, kind="ExternalInput")
    hs8_h = nc.dram_tensor("hs8", [128, NH // 2, 2, S - 512], F8,
                           kind="ExternalInput")
    wqf_h = nc.dram_tensor("wqf", [128, NH, QH_L * 128], BF16,
                           kind="ExternalInput")
    wkf_h = nc.dram_tensor("wkf", [128, NH, 128], BF16, kind="ExternalInput")
    wvf_h = nc.dram_tensor("wvf", [128, NH, 128], BF16, kind="ExternalInput")
    wq8_h = nc.dram_tensor("wq8", [128, NH // 2, 2, QH_L * 128], F8,
                           kind="ExternalInput")
    wk8_h = nc.dram_tensor("wk8", [128, NH // 2, 2, 128], F8,
                           kind="ExternalInput")
    wv8_h = nc.dram_tensor("wv8", [128, NH // 2, 2, 128], F8,
                           kind="ExternalInput")
    wob_h = nc.dram_tensor("wob", [128, DT, HID], BF16, kind="ExternalInput")
    csb_h = nc.dram_tensor("csb", [64, S], BF16, kind="ExternalInput")
    snb_h = nc.dram_tensor("snb", [64, S], BF16, kind="ExternalInput")
    msk0_h = nc.dram_tensor("msk0", [n0, 128, 512], BF16, kind="ExternalInput")
    msk8_h = nc.dram_tensor("msk8", [n8, 64, 2, 512], F8, kind="ExternalInput")
    id8_h = nc.dram_tensor("id8", [64, 2, 128], F8, kind="ExternalInput")
    ones8_h = nc.dram_tensor("ones8", [128, 2, 32], F8, kind="ExternalInput")
    identr_h = nc.dram_tensor("identr", [128, 128], F32R, kind="ExternalInput")
    identb_h = nc.dram_tensor("identb", [128, 128], BF16, kind="ExternalInput")
    onesf_h = nc.dram_tensor("onesf", [128, 1], F32R, kind="ExternalInput")
    sink_h = nc.dram_tensor("sinkexp", [1, QH_L], F32, kind="ExternalInput")
    out_h = nc.dram_tensor("out", [S, HID], BF16, kind="ExternalOutput")
    if _DBG:
        dbg_qk8_h = nc.dram_tensor("dbg_qk8", [64, 2, S], F8,
                                   kind="ExternalOutput")
        dbg_kk8_h = nc.dram_tensor("dbg_kk8", [64, 2, S], F8,
                                   kind="ExternalOutput")
        dbg_sc_h = nc.dram_tensor("dbg_sc", [128, 2, 512], F32,
                                  kind="ExternalOutput")
        dbg_pr_h = nc.dram_tensor("dbg_pr", [128, 2, 512], F8,
                                  kind="ExternalOutput")
        dbg_dn_h = nc.dram_tensor("dbg_dn", [32, 512], F32,
                                  kind="ExternalOutput")
        dbg_v_h = nc.dram_tensor("dbg_v", [128, NT, VD], F8,
                                 kind="ExternalOutput")

    lp = nc.allow_low_precision(reason="hybrid fp8/bf16 kernel, validated")
    lp.__enter__()

    with tile.TileContext(nc) as tc:
        with (
            tc.tile_pool(name="consts", bufs=1) as cpool,
            tc.tile_pool(name="big", bufs=1) as bigpool,
            tc.tile_pool(name="small", bufs=3) as spool,
        ):
            # ---- constants (gpsimd SWDGE queue; tiny) ----
            identr = cpool.tile([128, 128], F32R)
            nc.gpsimd.dma_start(identr[:], identr_h[:])
            identb = cpool.tile([128, 128], BF16)
            nc.gpsimd.dma_start(identb[:], identb_h[:])
            onesf = cpool.tile([128, 1], F32R)
            nc.gpsimd.dma_start(onesf[:], onesf_h[:])
            ones8 = cpool.tile([128, 2, 32], F8)
            nc.gpsimd.dma_start(ones8[:], ones8_h[:])
            id8 = cpool.tile([64, 2, 128], F8)
            nc.gpsimd.dma_start(id8[:], id8_h[:])
            sinkexp = cpool.tile([1, QH_L], F32)
            nc.gpsimd.dma_start(sinkexp[:], sink_h[:])
            csb = cpool.tile([64, S], BF16)
            nc.gpsimd.dma_start(csb[:], csb_h[:])
            snb = cpool.tile([64, S], BF16)
            nc.gpsimd.dma_start(snb[:], snb_h[:])

            # ---- resident activations ----
            qTb = [bigpool.tile([128, 512], BF16, tag=f"qTb{h}",
                                name=f"qTb{h}") for h in range(QH_L)]
            kTb = bigpool.tile([128, 512], BF16, tag="kTb")
            qk8 = [bigpool.tile([64, 2, S], F8, tag=f"qk8_{h}",
                                name=f"qk8_{h}") for h in range(QH_L)]
            kk8 = bigpool.tile([64, 2, S], F8, tag="kk8")
            vsbf = bigpool.tile([128, 4, VD], F32R, tag="vsbf")
            vsb8h = bigpool.tile([128, NT, VD], F8, tag="vsb8h")
            vsb8l = bigpool.tile([128, NT, VD], F8, tag="vsb8l")
            oTb = [bigpool.tile([128, S], BF16, tag=f"oTb{h}",
                                name=f"oTb{h}") for h in range(QH_L)]
            wob = bigpool.tile([128, DT, HID], BF16, tag="wob")

            # ================= phase A =================
            # resident inputs; fine chunks early so compute starts ASAP
            hsx0b = bigpool.tile([128, NH, 512], BF16, tag="hsx0b")
            for ch in range(4):
                nc.sync.dma_start(hsx0b[:, 4 * ch:4 * (ch + 1), :],
                                  hsx0_h[:, 4 * ch:4 * (ch + 1), :])
            hs8sb = bigpool.tile([128, NH // 2, 2, S - 512], F8, tag="hs8sb")
            for jc in range(2):
                nc.sync.dma_start(hs8sb[:, :, :, 512 * jc:512 * (jc + 1)],
                                  hs8_h[:, :, :, 512 * jc:512 * (jc + 1)])
            wqfb = bigpool.tile([128, NH, QH_L * 128], BF16, tag="wqfb")
            for t in range(4):
                nc.scalar.dma_start(wqfb[:, t, :], wqf_h[:, t, :])
            for ch in range(1, 4):
                nc.scalar.dma_start(wqfb[:, 4 * ch:4 * (ch + 1), :],
                                    wqf_h[:, 4 * ch:4 * (ch + 1), :])
            nc.scalar.dma_start(hs8sb[:, :, :, 1024:1536],
                                hs8_h[:, :, :, 1024:1536])
            wkfb = bigpool.tile([128, NH, 128], BF16, tag="wkfb")
            nc.gpsimd.dma_start(wkfb[:], wkf_h[:])
            wvfb = bigpool.tile([128, NH, 128], BF16, tag="wvfb")
            nc.gpsimd.dma_start(wvfb[:], wvf_h[:])
            msk0 = []
            for m in range(n0):
                t = cpool.tile([128, 512], BF16, tag=f"m0_{m}", name=f"m0_{m}")
                nc.gpsimd.dma_start(t[:], msk0_h[m])
                msk0.append(t)
            msk8 = []
            for m in range(n8):
                t = cpool.tile([64, 2, 512], F8, tag=f"m8_{m}", name=f"m8_{m}")
                nc.gpsimd.dma_start(t[:], msk8_h[m])
                msk8.append(t)
            with (
                tc.tile_pool(name="dstscr", bufs=3) as dpool,
                tc.tile_pool(name="psA", bufs=7, space="PSUM") as psA,
                tc.tile_pool(name="psX", bufs=1, space="PSUM") as psX,
            ):
                def rope_and_store(pp, hh, j, jsl):
                    """pp: psum [128,512] fp32 of q-head hh (or k if hh==4).
                    Writes bf16 (strip0) / fp8 planes (j>=1)."""
                    if j == 0:
                        dst = qTb[hh] if hh < QH_L else kTb
                        dsl = slice(0, 512)
                    else:
                        dst = dpool.tile([128, 512], BF16, tag="dst",
                                         name=f"dst{j}_{hh}")
                        dsl = slice(0, 512)
                    nc.scalar.copy(dst[:, dsl], pp[:])
                    swp = spool.tile([64, 512], BF16, tag="swp")
                    nc.sync.dma_start(swp[0:32, :], dst[32:64, dsl])
                    nc.sync.dma_start(swp[32:64, :], dst[0:32, dsl])
                    m1 = spool.tile([64, 512], BF16, tag="m1")
                    nc.vector.tensor_tensor(m1[:], dst[0:64, dsl],
                                            csb[:, jsl], ALU.mult)
                    m2 = spool.tile([64, 512], BF16, tag="m2")
                    nc.vector.tensor_tensor(m2[:], swp[:], snb[:, jsl],
                                            ALU.mult)
                    if j == 0:
                        nc.vector.tensor_tensor(dst[0:64, dsl], m1[:], m2[:],
                                                ALU.add)
                        if hh == QH_L:   # k head: also fp8 planes for keys<512
                            nc.vector.tensor_copy(kk8[:, 0, jsl],
                                                  dst[0:64, dsl])
                            scr = spool.tile([128, 512], F8, tag="scr")
                            nc.vector.tensor_copy(scr[64:128, :],
                                                  dst[64:128, dsl])
                            nc.sync.dma_start(kk8[:, 1, jsl], scr[64:128, :])
                    else:
                        t8 = qk8[hh] if hh < QH_L else kk8
                        nc.vector.tensor_tensor(t8[:, 0, jsl], m1[:], m2[:],
                                                ALU.add)
                        scr = spool.tile([128, 512], F8, tag="scr")
                        nc.vector.tensor_copy(scr[64:128, :], dst[64:128, dsl])
                        nc.sync.dma_start(t8[:, 1, jsl], scr[64:128, :])

                def v_store(pp, j):
                    vt = dpool.tile([128, 512], F32R, tag="vt",
                                    name=f"vt{j}")
                    nc.scalar.copy(vt[:], pp[:])
                    for st in range(4):
                        kt = 4 * j + st
                        tr = psX.tile([128, 128], F32R, tag="tr",
                                      name=f"tr{j}_{st}")
                        nc.tensor.transpose(
                            tr[:], vt[:, st * 128:(st + 1) * 128], identr[:]
                        )
                        if j == 0:
                            nc.vector.tensor_copy(vsbf[:, kt, :], tr[:])
                        nc.vector.tensor_copy(vsb8h[:, kt, :], tr[:])
                        tm = spool.tile([128, 128], F32, tag="vtm")
                        nc.vector.tensor_tensor(tm[:], tr[:],
                                                vsb8h[:, kt, :], ALU.subtract)
                        nc.vector.tensor_copy(vsb8l[:, kt, :], tm[:])

                # ---- strip 0: fp32r ----
                jsl0 = slice(0, 512)
                # pass 1: q heads (needs only wqfb chunks + hsx0b)
                pp0 = [psA.tile([128, 512], F32, tag="pp", name=f"pp0_{g}")
                       for g in range(QH_L)]
                for t in range(NH):
                    for g in range(QH_L):
                        nc.tensor.matmul(
                            pp0[g][:], wqfb[:, t, g * 128:(g + 1) * 128],
                            hsx0b[:, t, :],
                            start=(t == 0), stop=(t == NH - 1))
                for hh in range(QH_L):
                    rope_and_store(pp0[hh], hh, 0, jsl0)
                # pass 2: k and v
                ppk = psA.tile([128, 512], F32, tag="pp", name="pp0_k")
                ppv = psA.tile([128, 512], F32, tag="pp", name="pp0_v")
                for t in range(NH):
                    nc.tensor.matmul(ppk[:], wkfb[:, t, :], hsx0b[:, t, :],
                                     start=(t == 0), stop=(t == NH - 1))
                    nc.tensor.matmul(ppv[:], wvfb[:, t, :], hsx0b[:, t, :],
                                     start=(t == 0), stop=(t == NH - 1))
                rope_and_store(ppk, QH_L, 0, jsl0)
                v_store(ppv, 0)

                # ---- strips 1..3: fp8 DoubleRow ----
                wq8 = bigpool.tile([128, NH // 2, 2, QH_L * 128], F8,
                                   tag="wq8")
                nc.gpsimd.dma_start(wq8[:], wq8_h[:])
                wk8 = bigpool.tile([128, NH // 2, 2, 128], F8, tag="wk8")
                nc.gpsimd.dma_start(wk8[:], wk8_h[:])
                wv8 = bigpool.tile([128, NH // 2, 2, 128], F8, tag="wv8")
                nc.gpsimd.dma_start(wv8[:], wv8_h[:])
                # o-proj weights: needed only in phase C; load in background
                nc.gpsimd.dma_start(wob[:], wob_h[:])
                for j in range(1, NJ):
                    jsl = slice(512 * j, 512 * (j + 1))
                    rsl = slice(512 * (j - 1), 512 * j)  # hs8 is offset by 512
                    pp = [psA.tile([128, 512], F32, tag="pp",
                                   name=f"pp{j}_{g}")
                          for g in range(QH_L + 2)]
                    for t in range(NH // 2):
                        for g in range(QH_L + 2):
                            lhsT = (wq8[:, t, :, g * 128:(g + 1) * 128]
                                    if g < QH_L else
                                    (wk8[:, t, :, :] if g == QH_L
                                     else wv8[:, t, :, :]))
                            nc.tensor.matmul(pp[g][:], lhsT,
                                             hs8sb[:, t, :, rsl],
                                             start=(t == 0),
                                             stop=(t == NH // 2 - 1),
                                             perf_mode=DR)
                    for hh in range(QH_L + 1):
                        rope_and_store(pp[hh], hh, j, jsl)
                    v_store(pp[QH_L + 1], j)

            if _DBG:
                nc.sync.dma_start(dbg_qk8_h[:], qk8[0][:])
                nc.sync.dma_start(dbg_kk8_h[:], kk8[:])
                nc.sync.dma_start(dbg_v_h[:], vsb8h[:])

            # ================= phases B + C =================
            with (
                tc.tile_pool(name="pr8p", bufs=4) as pr8pool,
                tc.tile_pool(name="prfp", bufs=3) as prfpool,
                tc.tile_pool(name="osbp", bufs=2) as osbpool,
                tc.tile_pool(name="psSC", bufs=2, space="PSUM") as psSC,
                tc.tile_pool(name="psO", bufs=2, space="PSUM") as psO,
                tc.tile_pool(name="psDN", bufs=2, space="PSUM") as psDN,
            ):
                def emit_tail(tail):
                    dnrow, oacc, h, jsl_ = tail
                    dns = spool.tile([1, 512], F32, tag="dns")
                    nc.vector.tensor_scalar_add(dns[:], dnrow,
                                                sinkexp[0:1, h:h + 1])
                    recip = spool.tile([1, 512], F32, tag="recip")
                    nc.vector.reciprocal_approx_fast(recip[:], dns[:])
                    bc = spool.tile([128, 512], F32, tag="bc", bufs=2)
                    nc.gpsimd.partition_broadcast(bc[:], recip[:])
                    nc.vector.tensor_tensor(oTb[h][:, jsl_], oacc[:], bc[:],
                                            ALU.mult)

                def emit_b_strip(j):
                    jsl = slice(512 * j, 512 * (j + 1))
                    ktmax = 4 if j == 0 else NT
                    kts = [kt for kt in range(ktmax) if sched[j][kt] != SKIP]
                    prs = [kts[i:i + 2] for i in range(0, len(kts), 2)]
                    for pair_i, (ha, hb) in enumerate(((0, 1), (2, 3))):
                        st = {}
                        for h in (ha, hb):
                            st[h] = {
                                "oacc": psO.tile([128, 512], F32, tag="oacc",
                                                 name=f"oacc{j}_{h}"),
                                "dn": psDN.tile([32, 512], F32, tag="dn",
                                                name=f"dn{j}_{pair_i}_{h}"),
                                "first": True,
                            }

                        def emit_pv(pend, last):
                            grp, prt = pend
                            full = (j != 0 and len(grp) == 2
                                    and grp[1] == grp[0] + 1)
                            if j == 0:
                                for z, kt in enumerate(grp):
                                    lst = last and z == len(grp) - 1
                                    for h in (ha, hb):
                                        nc.tensor.matmul(
                                            st[h]["oacc"][:],
                                            vsbf[:, kt, :], prt[h][:, z, :],
                                            start=st[h]["first"], stop=lst)
                                    for h in (ha, hb):
                                        nc.tensor.matmul(
                                            st[h]["dn"][0:1, :],
                                            onesf[:], prt[h][:, z, :],
                                            start=st[h]["first"], stop=lst,
                                            skip_group_check=True)
                                        if lst or True:
                                            pass
                                    for h in (ha, hb):
                                        st[h]["first"] = False
                            elif full:
                                k0 = grp[0]
                                for h in (ha, hb):
                                    nc.tensor.matmul(
                                        st[h]["oacc"][:],
                                        vsb8h[:, k0:k0 + 2, :], prt[h][:],
                                        start=st[h]["first"], stop=False,
                                        perf_mode=DR)
                                for h in (ha, hb):
                                    nc.tensor.matmul(
                                        st[h]["oacc"][:],
                                        vsb8l[:, k0:k0 + 2, :], prt[h][:],
                                        start=False, stop=last,
                                        perf_mode=DR)
                                for h in (ha, hb):
                                    nc.tensor.matmul(
                                        st[h]["dn"][0:32, :],
                                        ones8[:], prt[h][:],
                                        start=st[h]["first"], stop=last,
                                        perf_mode=DR, skip_group_check=True)
                                for h in (ha, hb):
                                    st[h]["first"] = False
                            else:
                                for z, kt in enumerate(grp):
                                    lst = last and z == len(grp) - 1
                                    for h in (ha, hb):
                                        nc.tensor.matmul(
                                            st[h]["oacc"][:],
                                            vsb8h[:, kt, :], prt[h][:, z, :],
                                            start=st[h]["first"], stop=False)
                                    for h in (ha, hb):
                                        nc.tensor.matmul(
                                            st[h]["oacc"][:],
                                            vsb8l[:, kt, :], prt[h][:, z, :],
                                            start=False, stop=lst)
                                    for h in (ha, hb):
                                        nc.tensor.matmul(
                                            st[h]["dn"][0:32, :],
                                            ones8[:, 0, :], prt[h][:, z, :],
                                            start=st[h]["first"], stop=lst,
                                            skip_group_check=True)
                                    for h in (ha, hb):
                                        st[h]["first"] = False

                        pend = None
                        for pi, grp in enumerate(prs):
                            prt = {}
                            for h in (ha, hb):
                                sc2 = psSC.tile([128, 2, 512], F32,
                                                tag="sc2",
                                                name=f"sc{j}_{pair_i}_{h}_{pi}")
                                for z, kt in enumerate(grp):
                                    ksl = slice(kt * 128, (kt + 1) * 128)
                                    code = sched[j][kt]
                                    if j == 0:
                                        nc.tensor.matmul(
                                            sc2[:, z, :], kTb[:, ksl],
                                            qTb[h][:], start=True,
                                            stop=(code < 0))
                                        if code >= 0:
                                            nc.tensor.matmul(
                                                sc2[:, z, :], identb[:],
                                                msk0[code][:],
                                                start=False, stop=True)
                                    else:
                                        nc.tensor.matmul(
                                            sc2[:, z, :], kk8[:, :, ksl],
                                            qk8[h][:, :, jsl],
                                            start=True, stop=(code < 0),
                                            perf_mode=DR)
                                        if code >= 0:
                                            nc.tensor.matmul(
                                                sc2[:, z, :], id8[:],
                                                msk8[code][:],
                                                start=False, stop=True,
                                                perf_mode=DR)
                                if j == 0:
                                    pr = prfpool.tile([128, 2, 512], F32R,
                                                      tag="prf")
                                else:
                                    pr = pr8pool.tile([128, 2, 512], F8,
                                                      tag="pr8")
                                nc.scalar.activation(
                                    pr[:, 0:len(grp), :],
                                    sc2[:, 0:len(grp), :],
                                    AF.Exp, scale=SCALE)
                                prt[h] = pr
                            if pend is not None:
                                emit_pv(pend, last=False)
                            pend = (grp, prt)
                        if pend is not None:
                            emit_pv(pend, last=True)
                        for h in (ha, hb):
                            if st[h]["first"]:   # no valid kt at all
                                nc.vector.memset(oTb[h][:, jsl], 0.0)
                                continue
                            emit_tail((st[h]["dn"][0:1, :],
                                       st[h]["oacc"], h, jsl))

                emit_b_strip(0)
                emit_b_strip(1)
                emit_b_strip(2)
                emit_b_strip(3)

            # phase C after B pools close: use a wide PSUM pool
            with (
                tc.tile_pool(name="osb2", bufs=3) as osbpool,
                tc.tile_pool(name="psC2", bufs=8, space="PSUM") as psC2,
            ):
                for qt in range(0, 16):
                    qsl = slice(qt * 128, (qt + 1) * 128)
                    osb = osbpool.tile([128, HID], BF16, tag="osb",
                                       name=f"osbf{qt}")
                    ocs = [psC2.tile([128, 512], F32, tag="oc",
                                     name=f"oc{qt}_{hc}")
                           for hc in range(HID // 512)]
                    for t in range(DT):
                        for hc in range(HID // 512):
                            nc.tensor.matmul(
                                ocs[hc][:], oTb[t][:, qsl],
                                wob[:, t, hc * 512:(hc + 1) * 512],
                                start=(t == 0), stop=(t == DT - 1))
                    for hc in range(HID // 512):
                        hsl = slice(hc * 512, (hc + 1) * 512)
                        if hc % 2 == 0:
                            nc.scalar.copy(osb[:, hsl], ocs[hc][:])
                        else:
                            nc.vector.tensor_copy(osb[:, hsl], ocs[hc][:])
                    if qt % 2 == 0:
                        nc.sync.dma_start(out_h[qsl, :], osb[:])
                    else:
                        nc.gpsimd.dma_start(out_h[qsl, :], osb[:])

    lp.__exit__(None, None, None)
    nc.compile()
    return nc


def _classify_mask(mask):
    """Classify 512x128 blocks (strip j, k tile kt). Strip 0 gets fp32 mask
    tiles in raw-score units (mask/SCALE); strips >=1 get fp8 plane tiles."""
    sched = [[PLAIN] * NT for _ in range(NJ)]
    tiles0, seen0 = [], {}
    tiles8, seen8 = [], {}
    for j in range(NJ):
        for kt in range(NT):
            blk = mask[512 * j:512 * (j + 1), 128 * kt:128 * (kt + 1)]
            if np.all(blk <= -1e8):
                sched[j][kt] = SKIP
            elif not blk.any():
                sched[j][kt] = PLAIN
            else:
                key = blk.tobytes()
                if j == 0:
                    idx = seen0.get(key)
                    if idx is None:
                        idx = len(tiles0)
                        seen0[key] = idx
                        tiles0.append(
                            np.ascontiguousarray(blk.T / SCALE,
                                                 dtype=np.float32))
                    sched[j][kt] = idx
                else:
                    idx = seen8.get(key)
                    if idx is None:
                        idx = len(tiles8)
                        seen8[key] = idx
                        t = np.clip(blk.T / SCALE, -240.0, 240.0)
                        tiles8.append(
                            np.ascontiguousarray(
                                t.reshape(2, 64, 512).transpose(1, 0, 2)
                            ).astype(E4))
                    sched[j][kt] = idx
    m0 = (np.stack(tiles0) if tiles0
          else np.zeros((1, 128, 512), np.float32))
    m8 = (np.stack(tiles8) if tiles8
          else np.zeros((1, 64, 2, 512), E4))
    return sched, m0, m8


def _pt_layout(a, p=128):
    """[T*p, M] -> [p, T, M] partition-major tiling along the first axis."""
    t = a.shape[0] // p
    return np.ascontiguousarray(
        a.reshape(t, p, a.shape[1]).transpose(1, 0, 2), dtype=np.float32
    )


def kernel(**inputs):
    hs = np.asarray(inputs["hidden_states"], dtype=np.float32)[0]
    cos = np.asarray(inputs["cos"], dtype=np.float32)[0]
    sin = np.asarray(inputs["sin"], dtype=np.float32)[0]
    mask = np.asarray(inputs["attention_mask"], dtype=np.float32)[0, 0]
    Wq = np.asarray(inputs["Wq"], dtype=np.float32)
    Wk = np.asarray(inputs["Wk"], dtype=np.float32)
    Wv = np.asarray(inputs["Wv"], dtype=np.float32)
    Wo = np.asarray(inputs["Wo"], dtype=np.float32)
    sink = np.asarray(inputs["sink_bias"], dtype=np.float32)

    sched, m0, m8 = _classify_mask(mask)
    key = tuple(tuple(r) for r in sched)
    if key not in _cache:
        _cache[key] = _build(sched)
    nc = _cache[key]

    hsx = _pt_layout(hs.T)                           # [128, NH, S] fp32
    hsx0 = np.ascontiguousarray(hsx[:, :, 0:512]).astype(BF)
    hs8 = np.ascontiguousarray(
        hsx[:, :, 512:].reshape(128, NH // 2, 2, S - 512).astype(E4))

    csT = np.ascontiguousarray(cos.T)                # [64, S]
    snT = np.ascontiguousarray(sin.T).copy()
    snT[0:32] = -snT[0:32]                           # fold rotate-half sign
    csb = csT.astype(BF)
    snb = snT.astype(BF)

    id8 = np.zeros((64, 2, 128), E4)
    for i in range(64):
        id8[i, 0, i] = 1.0
        id8[i, 1, 64 + i] = 1.0
    ones8 = np.ones((128, 2, 32), E4)
    identr = np.eye(128, dtype=np.float32)
    onesf = np.ones((128, 1), np.float32)

    common = {
        "hsx0": hsx0, "hs8": hs8, "csb": csb, "snb": snb,
        "msk0": m0.astype(BF), "msk8": m8, "id8": id8, "ones8": ones8,
        "identr": identr, "identb": identr.astype(BF), "onesf": onesf,
        "wob": None,
    }

    in_maps = []
    for i in range(N_CORES):
        wqf = _pt_layout(np.ascontiguousarray(Wq[i * 512:(i + 1) * 512].T))
        wkf = _pt_layout(np.ascontiguousarray(Wk[i * 128:(i + 1) * 128].T))
        wvf = _pt_layout(np.ascontiguousarray(Wv[i * 128:(i + 1) * 128].T))
        wof = _pt_layout(np.ascontiguousarray(Wo[:, i * 512:(i + 1) * 512].T))
        se = np.exp(sink[i * QH_L:(i + 1) * QH_L]).reshape(1, QH_L)
        m = dict(common)
        m["wqf"] = wqf.astype(BF)
        m["wkf"] = wkf.astype(BF)
        m["wvf"] = wvf.astype(BF)
        m["wq8"] = np.ascontiguousarray(
            wqf.reshape(128, NH // 2, 2, 512)).astype(E4)
        m["wk8"] = np.ascontiguousarray(
            wkf.reshape(128, NH // 2, 2, 128)).astype(E4)
        m["wv8"] = np.ascontiguousarray(
            wvf.reshape(128, NH // 2, 2, 128)).astype(E4)
        m["wob"] = wof.astype(BF)
        m["sinkexp"] = np.ascontiguousarray(se, dtype=np.float32)
        in_maps.append(m)

    global _last
    _last = (nc, in_maps)
    res = run_bass_kernel_spmd(nc, in_maps, list(range(N_CORES)))
    out = np.zeros((S, HID), np.float64)
    for i in range(N_CORES):
        out += res.results[i]["out"].astype(np.float64)
    out = out.astype(np.float32).reshape(B, S, HID)
    if not np.isfinite(out).all():
        raise FloatingPointError(
            "kernel produced non-finite values; inputs outside the "
            "validated regime for the no-max-pass softmax"
        )
    return out


# revision 21
# speedup vs baseline: 1.0719x; 1.0627x over previous
"""Trainium2 Bass kernel for nn_MiMoV2FlashBlock (GQA attention block with
partial RoPE and attention-sink softmax), sharded across 8 NeuronCores.

Sharding: tensor-parallel over heads. Core i computes q-heads 4i..4i+3 and
kv-head i, plus the matching input-dim shard of the o-projection. Each core
emits a partial [S, HID] output (bf16); the host sums the 8 partials in fp64.

Precision strategy (strip-hybrid): query rows < 512 ("strip 0") take an
exact path (fp32 projections, bf16 q/k, fp32r attention); rows >= 512 run on
the fp8e4m3 DoubleRow path (4x PE throughput) where softmax averaging over a
large attention support washes out quantization noise:
  - projections for positions >= 512: fp8 hs x fp8 W, DoubleRow pairs.
  - scores: fp8 q/k split into two 64-partition d-planes, DoubleRow.
  - probs: exp -> fp8 directly; denominator via all-ones DoubleRow matmul of
    the SAME fp8 probs (quantization cancels in the ratio).
  - PV: DoubleRow with v in two fp8 terms (hi + residual) for accuracy.
  - causal masking for fp8 blocks: identity-DoubleRow accumulate of the mask
    (in raw-score units, clamped to +-240 = e4m3 max finite) into the scores
    PSUM before exp; exp then flushes masked entries to 0.
o-projection runs in bf16. RoPE's rotate-half uses an SBUF->SBUF DMA
partition swap (sign folded into the sin table) instead of PE matmuls.
Validated end-to-end against the fp32 reference: rel err ~8e-3 (gate 2e-2).
"""

import sys

for _p in ("/opt/trn_rl_repo",):
    if _p not in sys.path:
        sys.path.insert(0, _p)

import numpy as np
import ml_dtypes

import concourse.bass as bass  # noqa: E402
import concourse.mybir as mybir  # noqa: E402
import concourse.tile as tile  # noqa: E402
from concourse import bacc  # noqa: E402
from concourse import hw_specs as _hw_specs  # noqa: E402
from concourse.bass_utils import run_bass_kernel_spmd  # noqa: E402

# Pin activation-table selection to the set containing Exp, Ln and Copy so the
# scheduler never pays a mid-kernel ACT_TABLE_LOAD.
_orig_gat = _hw_specs.get_activation_tables


def _pinned_act_tables(arch):
    full = _orig_gat(arch)
    return {
        k: (v if k == "natural_log_exp_and_others" else set())
        for k, v in full.items()
    }


bacc.get_activation_tables = _pinned_act_tables

F32 = mybir.dt.float32
F32R = mybir.dt.float32r
BF16 = mybir.dt.bfloat16
F8 = mybir.dt.float8e4
AF = mybir.ActivationFunctionType
ALU = mybir.AluOpType
DR = mybir.MatmulPerfMode.DoubleRow
E4 = ml_dtypes.float8_e4m3
BF = ml_dtypes.bfloat16

B, S, HID = 1, 2048, 2048
H, KVH, D, VD = 32, 8, 128, 128
R = 64
N_CORES = 8
QH_L = H // N_CORES          # 4 local q heads per core
SCALE = float(D) ** -0.5

NT = S // 128                # 16 k tiles
NH = HID // 128              # 16 hidden tiles
NJ = S // 512                # 4 q strips
DT = (QH_L * VD) // 128      # 4 o-proj contraction tiles

SKIP, PLAIN = -1, -2
_DBG = False

_cache: dict = {}


def _build(sched):
    """sched[j][kt] in {SKIP, PLAIN, idx>=0}. Strip 0 mask tiles are fp32
    (idx into msk0); strips >=1 are fp8 plane tiles (idx into msk8)."""
    n0 = max(1 + max((sched[0][kt] for kt in range(NT)), default=-1), 1)
    n8 = max(max((sched[j][kt] for kt in range(NT)), default=-1)
             for j in range(1, NJ)) + 1
    n8 = max(n8, 1)

    nc = bacc.Bacc(None, target_bir_lowering=False)

    hsx0_h = nc.dram_tensor("hsx0", [128, NH, 512], BF16, kind="ExternalInput")
    hs8_h = nc.dram_tensor("hs8", [128, NH // 2, 2, S - 512], F8,
                           kind="ExternalInput")
    wqf_h = nc.dram_tensor("wqf", [128, NH, QH_L * 128], BF16,
                           kind="ExternalInput")
    wkf_h = nc.dram_tensor("wkf", [128, NH, 128], BF16, kind="ExternalInput")
    wvf_h = nc.dram_tensor("wvf", [128, NH, 128], BF16, kind="ExternalInput")
    wq8_h = nc.dram_tensor("wq8", [128, NH // 2, 2, QH_L * 128], F8,
                           kind="ExternalInput")
    wk8_h = nc.dram_tensor("wk8", [128, NH // 2, 2, 128], F8,
                           kind="ExternalInput")
    wv8_h = nc.dram_tensor("wv8", [128, NH // 2, 2, 128], F8,
                           kind="ExternalInput")
    wob_h = nc.dram_tensor("wob", [128, DT, HID], BF16, kind="ExternalInput")
    csb_h = nc.dram_tensor("csb", [64, S], BF16, kind="ExternalInput")
    snb_h = nc.dram_tensor("snb", [64, S], BF16, kind="ExternalInput")
    msk0_h = nc.dram_tensor("msk0", [n0, 128, 512], BF16, kind="ExternalInput")
    msk8_h = nc.dram_tensor("msk8", [n8, 64, 2, 512], F8, kind="ExternalInput")
    id8_h = nc.dram_tensor("id8", [64, 2, 128], F8, kind="ExternalInput")
    ones8_h = nc.dram_tensor("ones8", [128, 2, 32], F8, kind="ExternalInput")
    identr_h = nc.dram_tensor("identr", [128, 128], F32R, kind="ExternalInput")
    identb_h = nc.dram_tensor("identb", [128, 128], BF16, kind="ExternalInput")
    onesf_h = nc.dram_tensor("onesf", [128, 1], F32R, kind="ExternalInput")
    sink_h = nc.dram_tensor("sinkexp", [1, QH_L], F32, kind="ExternalInput")
    out_h = nc.dram_tensor("out", [S, HID], BF16, kind="ExternalOutput")
    if _DBG:
        dbg_qk8_h = nc.dram_tensor("dbg_qk8", [64, 2, S], F8,
                                   kind="ExternalOutput")
        dbg_kk8_h = nc.dram_tensor("dbg_kk8", [64, 2, S], F8,
                                   kind="ExternalOutput")
        dbg_sc_h = nc.dram_tensor("dbg_sc", [128, 2, 512], F32,
                                  kind="ExternalOutput")
        dbg_pr_h = nc.dram_tensor("dbg_pr", [128, 2, 512], F8,
                                  kind="ExternalOutput")
        dbg_dn_h = nc.dram_tensor("dbg_dn", [32, 512], F32,
                                  kind="ExternalOutput")
        dbg_v_h = nc.dram_tensor("dbg_v", [128, NT, VD], F8,
                                 kind="ExternalOutput")

    lp = nc.allow_low_precision(reason="hybrid fp8/bf16 kernel, validated")
    lp.__enter__()

    with tile.TileContext(nc) as tc:
        with (
            tc.tile_pool(name="consts", bufs=1) as cpool,
            tc.tile_pool(name="big", bufs=1) as bigpool,
            tc.tile_pool(name="small", bufs=3) as spool,
        ):
            # ---- constants (gpsimd SWDGE queue; tiny) ----
            identr = cpool.tile([128, 128], F32R)
            nc.gpsimd.dma_start(identr[:], identr_h[:])
            identb = cpool.tile([128, 128], BF16)
            nc.gpsimd.dma_start(identb[:], identb_h[:])
            onesf = cpool.tile([128, 1], F32R)
            nc.gpsimd.dma_start(onesf[:], onesf_h[:])
            ones8 = cpool.tile([128, 2, 32], F8)
            nc.gpsimd.dma_start(ones8[:], ones8_h[:])
            id8 = cpool.tile([64, 2, 128], F8)
            nc.gpsimd.dma_start(id8[:], id8_h[:])
            sinkexp = cpool.tile([1, QH_L], F32)
            nc.gpsimd.dma_start(sinkexp[:], sink_h[:])
            csb = cpool.tile([64, S], BF16)
            nc.gpsimd.dma_start(csb[:], csb_h[:])
            snb = cpool.tile([64, S], BF16)
            nc.gpsimd.dma_start(snb[:], snb_h[:])

            # ---- resident activations ----
            qTb = [bigpool.tile([128, 512], BF16, tag=f"qTb{h}",
                                name=f"qTb{h}") for h in range(QH_L)]
            kTb = bigpool.tile([128, 512], BF16, tag="kTb")
            qk8 = [bigpool.tile([64, 2, S], F8, tag=f"qk8_{h}",
                                name=f"qk8_{h}") for h in range(QH_L)]
            kk8 = bigpool.tile([64, 2, S], F8, tag="kk8")
            vsbf = bigpool.tile([128, 4, VD], F32R, tag="vsbf")
            vsb8h = bigpool.tile([128, NT, VD], F8, tag="vsb8h")
            oTb = [bigpool.tile([128, S], BF16, tag=f"oTb{h}",
                                name=f"oTb{h}") for h in range(QH_L)]
            wob = bigpool.tile([128, DT, HID], BF16, tag="wob")

            # ================= phase A =================
            # resident inputs; fine chunks early so compute starts ASAP
            hsx0b = bigpool.tile([128, NH, 512], BF16, tag="hsx0b")
            for ch in range(4):
                nc.sync.dma_start(hsx0b[:, 4 * ch:4 * (ch + 1), :],
                                  hsx0_h[:, 4 * ch:4 * (ch + 1), :])
            hs8sb = bigpool.tile([128, NH // 2, 2, S - 512], F8, tag="hs8sb")
            for jc in range(2):
                nc.sync.dma_start(hs8sb[:, :, :, 512 * jc:512 * (jc + 1)],
                                  hs8_h[:, :, :, 512 * jc:512 * (jc + 1)])
            wqfb = bigpool.tile([128, NH, QH_L * 128], BF16, tag="wqfb")
            for t in range(4):
                nc.scalar.dma_start(wqfb[:, t, :], wqf_h[:, t, :])
            for ch in range(1, 4):
                nc.scalar.dma_start(wqfb[:, 4 * ch:4 * (ch + 1), :],
                                    wqf_h[:, 4 * ch:4 * (ch + 1), :])
            nc.scalar.dma_start(hs8sb[:, :, :, 1024:1536],
                                hs8_h[:, :, :, 1024:1536])
            wkfb = bigpool.tile([128, NH, 128], BF16, tag="wkfb")
            nc.gpsimd.dma_start(wkfb[:], wkf_h[:])
            wvfb = bigpool.tile([128, NH, 128], BF16, tag="wvfb")
            nc.gpsimd.dma_start(wvfb[:], wvf_h[:])
            msk0 = []
            for m in range(n0):
                t = cpool.tile([128, 512], BF16, tag=f"m0_{m}", name=f"m0_{m}")
                nc.gpsimd.dma_start(t[:], msk0_h[m])
                msk0.append(t)
            msk8 = []
            for m in range(n8):
                t = cpool.tile([64, 2, 512], F8, tag=f"m8_{m}", name=f"m8_{m}")
                nc.gpsimd.dma_start(t[:], msk8_h[m])
                msk8.append(t)
            with (
                tc.tile_pool(name="dstscr", bufs=3) as dpool,
                tc.tile_pool(name="psA", bufs=7, space="PSUM") as psA,
                tc.tile_pool(name="psX", bufs=1, space="PSUM") as psX,
            ):
                def rope_and_store(pp, hh, j, jsl):
                    """pp: psum [128,512] fp32 of q-head hh (or k if hh==4).
                    Writes bf16 (strip0) / fp8 planes (j>=1)."""
                    if j == 0:
                        dst = qTb[hh] if hh < QH_L else kTb
                        dsl = slice(0, 512)
                    else:
                        dst = dpool.tile([128, 512], BF16, tag="dst",
                                         name=f"dst{j}_{hh}")
                        dsl = slice(0, 512)
                    nc.scalar.copy(dst[:, dsl], pp[:])
                    swp = spool.tile([64, 512], BF16, tag="swp")
                    nc.sync.dma_start(swp[0:32, :], dst[32:64, dsl])
                    nc.sync.dma_start(swp[32:64, :], dst[0:32, dsl])
                    m1 = spool.tile([64, 512], BF16, tag="m1")
                    nc.vector.tensor_tensor(m1[:], dst[0:64, dsl],
                                            csb[:, jsl], ALU.mult)
                    m2 = spool.tile([64, 512], BF16, tag="m2")
                    nc.vector.tensor_tensor(m2[:], swp[:], snb[:, jsl],
                                            ALU.mult)
                    if j == 0:
                        nc.vector.tensor_tensor(dst[0:64, dsl], m1[:], m2[:],
                                                ALU.add)
                        if hh == QH_L:   # k head: also fp8 planes for keys<512
                            nc.vector.tensor_copy(kk8[:, 0, jsl],
                                                  dst[0:64, dsl])
                            scr = spool.tile([128, 512], F8, tag="scr")
                            nc.vector.tensor_copy(scr[64:128, :],
                                                  dst[64:128, dsl])
                            nc.sync.dma_start(kk8[:, 1, jsl], scr[64:128, :])
                    else:
                        t8 = qk8[hh] if hh < QH_L else kk8
                        nc.vector.tensor_tensor(t8[:, 0, jsl], m1[:], m2[:],
                                                ALU.add)
                        scr = spool.tile([128, 512], F8, tag="scr")
                        nc.vector.tensor_copy(scr[64:128, :], dst[64:128, dsl])
                        nc.sync.dma_start(t8[:, 1, jsl], scr[64:128, :])

                def v_store(pp, j):
                    vt = dpool.tile([128, 512], F32R, tag="vt",
                                    name=f"vt{j}")
                    nc.scalar.copy(vt[:], pp[:])
                    for st in range(4):
                        kt = 4 * j + st
                        tr = psX.tile([128, 128], F32R, tag="tr",
                                      name=f"tr{j}_{st}")
                        nc.tensor.transpose(
                            tr[:], vt[:, st * 128:(st + 1) * 128], identr[:]
                        )
                        if j == 0:
                            nc.vector.tensor_copy(vsbf[:, kt, :], tr[:])
                        nc.vector.tensor_copy(vsb8h[:, kt, :], tr[:])

                # ---- strip 0: fp32r ----
                jsl0 = slice(0, 512)
                # pass 1: q heads (needs only wqfb chunks + hsx0b)
                pp0 = [psA.tile([128, 512], F32, tag="pp", name=f"pp0_{g}")
                       for g in range(QH_L)]
                for t in range(NH):
                    for g in range(QH_L):
                        nc.tensor.matmul(
                            pp0[g][:], wqfb[:, t, g * 128:(g + 1) * 128],
                            hsx0b[:, t, :],
                            start=(t == 0), stop=(t == NH - 1))
                for hh in range(QH_L):
                    rope_and_store(pp0[hh], hh, 0, jsl0)
                # pass 2: k and v
                ppk = psA.tile([128, 512], F32, tag="pp", name="pp0_k")
                ppv = psA.tile([128, 512], F32, tag="pp", name="pp0_v")
                for t in range(NH):
                    nc.tensor.matmul(ppk[:], wkfb[:, t, :], hsx0b[:, t, :],
                                     start=(t == 0), stop=(t == NH - 1))
                    nc.tensor.matmul(ppv[:], wvfb[:, t, :], hsx0b[:, t, :],
                                     start=(t == 0), stop=(t == NH - 1))
                rope_and_store(ppk, QH_L, 0, jsl0)
                v_store(ppv, 0)

                # ---- strips 1..3: fp8 DoubleRow ----
                wq8 = bigpool.tile([128, NH // 2, 2, QH_L * 128], F8,
                                   tag="wq8")
                nc.gpsimd.dma_start(wq8[:], wq8_h[:])
                wk8 = bigpool.tile([128, NH // 2, 2, 128], F8, tag="wk8")
                nc.gpsimd.dma_start(wk8[:], wk8_h[:])
                wv8 = bigpool.tile([128, NH // 2, 2, 128], F8, tag="wv8")
                nc.gpsimd.dma_start(wv8[:], wv8_h[:])
                # o-proj weights: needed only in phase C; load in background
                nc.gpsimd.dma_start(wob[:], wob_h[:])
                for j in range(1, NJ):
                    jsl = slice(512 * j, 512 * (j + 1))
                    rsl = slice(512 * (j - 1), 512 * j)  # hs8 is offset by 512
                    pp = [psA.tile([128, 512], F32, tag="pp",
                                   name=f"pp{j}_{g}")
                          for g in range(QH_L + 2)]
                    for t in range(NH // 2):
                        for g in range(QH_L + 2):
                            lhsT = (wq8[:, t, :, g * 128:(g + 1) * 128]
                                    if g < QH_L else
                                    (wk8[:, t, :, :] if g == QH_L
                                     else wv8[:, t, :, :]))
                            nc.tensor.matmul(pp[g][:], lhsT,
                                             hs8sb[:, t, :, rsl],
                                             start=(t == 0),
                                             stop=(t == NH // 2 - 1),
                                             perf_mode=DR)
                    for hh in range(QH_L + 1):
                        rope_and_store(pp[hh], hh, j, jsl)
                    v_store(pp[QH_L + 1], j)

            if _DBG:
                nc.sync.dma_start(dbg_qk8_h[:], qk8[0][:])
                nc.sync.dma_start(dbg_kk8_h[:], kk8[:])
                nc.sync.dma_start(dbg_v_h[:], vsb8h[:])

            # ================= phases B + C =================
            with (
                tc.tile_pool(name="pr8p", bufs=4) as pr8pool,
                tc.tile_pool(name="prfp", bufs=3) as prfpool,
                tc.tile_pool(name="osbp", bufs=2) as osbpool,
                tc.tile_pool(name="psSC", bufs=2, space="PSUM") as psSC,
                tc.tile_pool(name="psO", bufs=2, space="PSUM") as psO,
                tc.tile_pool(name="psDN", bufs=2, space="PSUM") as psDN,
            ):
                def emit_tail(tail):
                    dnrow, oacc, h, jsl_ = tail
                    dns = spool.tile([1, 512], F32, tag="dns")
                    nc.vector.tensor_scalar_add(dns[:], dnrow,
                                                sinkexp[0:1, h:h + 1])
                    recip = spool.tile([1, 512], F32, tag="recip")
                    nc.vector.reciprocal_approx_fast(recip[:], dns[:])
                    bc = spool.tile([128, 512], F32, tag="bc", bufs=2)
                    nc.gpsimd.partition_broadcast(bc[:], recip[:])
                    nc.vector.tensor_tensor(oTb[h][:, jsl_], oacc[:], bc[:],
                                            ALU.mult)

                def emit_b_strip(j):
                    jsl = slice(512 * j, 512 * (j + 1))
                    ktmax = 4 if j == 0 else NT
                    kts = [kt for kt in range(ktmax) if sched[j][kt] != SKIP]
                    prs = [kts[i:i + 2] for i in range(0, len(kts), 2)]
                    for pair_i, (ha, hb) in enumerate(((0, 1), (2, 3))):
                        st = {}
                        for h in (ha, hb):
                            st[h] = {
                                "oacc": psO.tile([128, 512], F32, tag="oacc",
                                                 name=f"oacc{j}_{h}"),
                                "dn": psDN.tile([32, 512], F32, tag="dn",
                                                name=f"dn{j}_{pair_i}_{h}"),
                                "first": True,
                            }

                        def emit_pv(pend, last):
                            grp, prt = pend
                            full = (j != 0 and len(grp) == 2
                                    and grp[1] == grp[0] + 1)
                            if j == 0:
                                for z, kt in enumerate(grp):
                                    lst = last and z == len(grp) - 1
                                    for h in (ha, hb):
                                        nc.tensor.matmul(
                                            st[h]["oacc"][:],
                                            vsbf[:, kt, :], prt[h][:, z, :],
                                            start=st[h]["first"], stop=lst)
                                    for h in (ha, hb):
                                        nc.tensor.matmul(
                                            st[h]["dn"][0:1, :],
                                            onesf[:], prt[h][:, z, :],
                                            start=st[h]["first"], stop=lst,
                                            skip_group_check=True)
                                        if lst or True:
                                            pass
                                    for h in (ha, hb):
                                        st[h]["first"] = False
                            elif full:
                                k0 = grp[0]
                                for h in (ha, hb):
                                    nc.tensor.matmul(
                                        st[h]["oacc"][:],
                                        vsb8h[:, k0:k0 + 2, :], prt[h][:],
                                        start=st[h]["first"], stop=last,
                                        perf_mode=DR)
                                for h in (ha, hb):
                                    nc.tensor.matmul(
                                        st[h]["dn"][0:32, :],
                                        ones8[:], prt[h][:],
                                        start=st[h]["first"], stop=last,
                                        perf_mode=DR, skip_group_check=True)
                                for h in (ha, hb):
                                    st[h]["first"] = False
                            else:
                                for z, kt in enumerate(grp):
                                    lst = last and z == len(grp) - 1
                                    for h in (ha, hb):
                                        nc.tensor.matmul(
                                            st[h]["oacc"][:],
                                            vsb8h[:, kt, :], prt[h][:, z, :],
                                            start=st[h]["first"], stop=lst)
                                    for h in (ha, hb):
                                        nc.tensor.matmul(
                                            st[h]["dn"][0:32, :],
                                            ones8[:, 0, :], prt[h][:, z, :],
                                            start=st[h]["first"], stop=lst,
                                            skip_group_check=True)
                                    for h in (ha, hb):
                                        st[h]["first"] = False

                        pend = None
                        for pi, grp in enumerate(prs):
                            prt = {}
                            for h in (ha, hb):
                                sc2 = psSC.tile([128, 2, 512], F32,
                                                tag="sc2",
                                                name=f"sc{j}_{pair_i}_{h}_{pi}")
                                for z, kt in enumerate(grp):
                                    ksl = slice(kt * 128, (kt + 1) * 128)
                                    code = sched[j][kt]
                                    if j == 0:
                                        nc.tensor.matmul(
                                            sc2[:, z, :], kTb[:, ksl],
                                            qTb[h][:], start=True,
                                            stop=(code < 0))
                                        if code >= 0:
                                            nc.tensor.matmul(
                                                sc2[:, z, :], identb[:],
                                                msk0[code][:],
                                                start=False, stop=True)
                                    else:
                                        nc.tensor.matmul(
                                            sc2[:, z, :], kk8[:, :, ksl],
                                            qk8[h][:, :, jsl],
                                            start=True, stop=(code < 0),
                                            perf_mode=DR)
                                        if code >= 0:
                                            nc.tensor.matmul(
                                                sc2[:, z, :], id8[:],
                                                msk8[code][:],
                                                start=False, stop=True,
                                                perf_mode=DR)
                                if j == 0:
                                    pr = prfpool.tile([128, 2, 512], F32R,
                                                      tag="prf")
                                else:
                                    pr = pr8pool.tile([128, 2, 512], F8,
                                                      tag="pr8")
                                nc.scalar.activation(
                                    pr[:, 0:len(grp), :],
                                    sc2[:, 0:len(grp), :],
                                    AF.Exp, scale=SCALE)
                                prt[h] = pr
                            if pend is not None:
                                emit_pv(pend, last=False)
                            pend = (grp, prt)
                        if pend is not None:
                            emit_pv(pend, last=True)
                        for h in (ha, hb):
                            if st[h]["first"]:   # no valid kt at all
                                nc.vector.memset(oTb[h][:, jsl], 0.0)
                                continue
                            emit_tail((st[h]["dn"][0:1, :],
                                       st[h]["oacc"], h, jsl))

                emit_b_strip(0)
                emit_b_strip(1)
                emit_b_strip(2)
                emit_b_strip(3)

            # phase C after B pools close: use a wide PSUM pool
            with (
                tc.tile_pool(name="osb2", bufs=3) as osbpool,
                tc.tile_pool(name="psC2", bufs=8, space="PSUM") as psC2,
            ):
                for qt in range(0, 16):
                    qsl = slice(qt * 128, (qt + 1) * 128)
                    osb = osbpool.tile([128, HID], BF16, tag="osb",
                                       name=f"osbf{qt}")
                    ocs = [psC2.tile([128, 512], F32, tag="oc",
                                     name=f"oc{qt}_{hc}")
                           for hc in range(HID // 512)]
                    for t in range(DT):
                        for hc in range(HID // 512):
                            nc.tensor.matmul(
                                ocs[hc][:], oTb[t][:, qsl],
                                wob[:, t, hc * 512:(hc + 1) * 512],
                                start=(t == 0), stop=(t == DT - 1))
                    for hc in range(HID // 512):
                        hsl = slice(hc * 512, (hc + 1) * 512)
                        if hc % 2 == 0:
                            nc.scalar.copy(osb[:, hsl], ocs[hc][:])
                        else:
                            nc.vector.tensor_copy(osb[:, hsl], ocs[hc][:])
                    if qt % 2 == 0:
                        nc.sync.dma_start(out_h[qsl, :], osb[:])
                    else:
                        nc.gpsimd.dma_start(out_h[qsl, :], osb[:])

    lp.__exit__(None, None, None)
    nc.compile()
    return nc


def _classify_mask(mask):
    """Classify 512x128 blocks (strip j, k tile kt). Strip 0 gets fp32 mask
    tiles in raw-score units (mask/SCALE); strips >=1 get fp8 plane tiles."""
    sched = [[PLAIN] * NT for _ in range(NJ)]
    tiles0, seen0 = [], {}
    tiles8, seen8 = [], {}
    for j in range(NJ):
        for kt in range(NT):
            blk = mask[512 * j:512 * (j + 1), 128 * kt:128 * (kt + 1)]
            if np.all(blk <= -1e8):
                sched[j][kt] = SKIP
            elif not blk.any():
                sched[j][kt] = PLAIN
            else:
                key = blk.tobytes()
                if j == 0:
                    idx = seen0.get(key)
                    if idx is None:
                        idx = len(tiles0)
                        seen0[key] = idx
                        tiles0.append(
                            np.ascontiguousarray(blk.T / SCALE,
                                                 dtype=np.float32))
                    sched[j][kt] = idx
                else:
                    idx = seen8.get(key)
                    if idx is None:
                        idx = len(tiles8)
                        seen8[key] = idx
                        t = np.clip(blk.T / SCALE, -240.0, 240.0)
                        tiles8.append(
                            np.ascontiguousarray(
                                t.reshape(2, 64, 512).transpose(1, 0, 2)
                            ).astype(E4))
                    sched[j][kt] = idx
    m0 = (np.stack(tiles0) if tiles0
          else np.zeros((1, 128, 512), np.float32))
    m8 = (np.stack(tiles8) if tiles8
          else np.zeros((1, 64, 2, 512), E4))
    return sched, m0, m8


def _pt_layout(a, p=128):
    """[T*p, M] -> [p, T, M] partition-major tiling along the first axis."""
    t = a.shape[0] // p
    return np.ascontiguousarray(
        a.reshape(t, p, a.shape[1]).transpose(1, 0, 2), dtype=np.float32
    )


def kernel(**inputs):
    hs = np.asarray(inputs["hidden_states"], dtype=np.float32)[0]
    cos = np.asarray(inputs["cos"], dtype=np.float32)[0]
    sin = np.asarray(inputs["sin"], dtype=np.float32)[0]
    mask = np.asarray(inputs["attention_mask"], dtype=np.float32)[0, 0]
    Wq = np.asarray(inputs["Wq"], dtype=np.float32)
    Wk = np.asarray(inputs["Wk"], dtype=np.float32)
    Wv = np.asarray(inputs["Wv"], dtype=np.float32)
    Wo = np.asarray(inputs["Wo"], dtype=np.float32)
    sink = np.asarray(inputs["sink_bias"], dtype=np.float32)

    sched, m0, m8 = _classify_mask(mask)
    key = tuple(tuple(r) for r in sched)
    if key not in _cache:
        _cache[key] = _build(sched)
    nc = _cache[key]

    hsx = _pt_layout(hs.T)                           # [128, NH, S] fp32
    hsx0 = np.ascontiguousarray(hsx[:, :, 0:512]).astype(BF)
    hs8 = np.ascontiguousarray(
        hsx[:, :, 512:].reshape(128, NH // 2, 2, S - 512).astype(E4))

    csT = np.ascontiguousarray(cos.T)                # [64, S]
    snT = np.ascontiguousarray(sin.T).copy()
    snT[0:32] = -snT[0:32]                           # fold rotate-half sign
    csb = csT.astype(BF)
    snb = snT.astype(BF)

    id8 = np.zeros((64, 2, 128), E4)
    for i in range(64):
        id8[i, 0, i] = 1.0
        id8[i, 1, 64 + i] = 1.0
    ones8 = np.ones((128, 2, 32), E4)
    identr = np.eye(128, dtype=np.float32)
    onesf = np.ones((128, 1), np.float32)

    common = {
        "hsx0": hsx0, "hs8": hs8, "csb": csb, "snb": snb,
        "msk0": m0.astype(BF), "msk8": m8, "id8": id8, "ones8": ones8,
        "identr": identr, "identb": identr.astype(BF), "onesf": onesf,
        "wob": None,
    }

    in_maps = []
    for i in range(N_CORES):
        wqf = _pt_layout(np.ascontiguousarray(Wq[i * 512:(i + 1) * 512].T))
        wkf = _pt_layout(np.ascontiguousarray(Wk[i * 128:(i + 1) * 128].T))
        wvf = _pt_layout(np.ascontiguousarray(Wv[i * 128:(i + 1) * 128].T))
        wof = _pt_layout(np.ascontiguousarray(Wo[:, i * 512:(i + 1) * 512].T))
        se = np.exp(sink[i * QH_L:(i + 1) * QH_L]).reshape(1, QH_L)
        m = dict(common)
        m["wqf"] = wqf.astype(BF)
        m["wkf"] = wkf.astype(BF)
        m["wvf"] = wvf.astype(BF)
        m["wq8"] = np.ascontiguousarray(
            wqf.reshape(128, NH // 2, 2, 512)).astype(E4)
        m["wk8"] = np.ascontiguousarray(
            wkf.reshape(128, NH // 2, 2, 128)).astype(E4)
        m["wv8"] = np.ascontiguousarray(
            wvf.reshape(128, NH // 2, 2, 128)).astype(E4)
        m["wob"] = wof.astype(BF)
        m["sinkexp"] = np.ascontiguousarray(se, dtype=np.float32)
        in_maps.append(m)

    global _last
    _last = (nc, in_maps)
    res = run_bass_kernel_spmd(nc, in_maps, list(range(N_CORES)))
    out = np.zeros((S, HID), np.float64)
    for i in range(N_CORES):
        out += res.results[i]["out"].astype(np.float64)
    out = out.astype(np.float32).reshape(B, S, HID)
    if not np.isfinite(out).all():
        raise FloatingPointError(
            "kernel produced non-finite values; inputs outside the "
            "validated regime for the no-max-pass softmax"
        )
    return out
